# revision 10
# baseline (speedup 1.0000x reference)
"""Tensor-parallel causal GQA multi-head attention for 8 TRN2 NeuronCores.

Problem (hardcoded): x (2,2048,2048) f32, wq (2048,2048), wk/wv (2048,512),
wo (2048,2048); 32 q heads, 8 kv heads, head_dim 64, interleaved RoPE
(base 10000), causal softmax, output projection.

Sharding: core c handles batch b=c//4 and head group g=c%4 (8 q heads,
2 kv heads). Each core computes y_partial = attn(heads of g) @ wo[rows g]
for its batch; the host sums the 4 partials per batch (no collectives).

Device algorithm (bf16 matmuls, fp32 PSUM accumulation):
  - projections computed d-major (dims on partitions): qT/kT = w.T @ x.T
    with w stationary, xT moving.  RoPE applied d-major with host-permuted
    weight columns ([even dims; odd dims] per head) and replicated cos/sin
    tables, so all ops are 32-row aligned DVE tensor ops.
  - scoresT (sk x sq) per head via K=64 matmuls, two heads packed in the
    128x128 PE array via tile_position row groups; two sk tiles per
    (128,1024) PSUM pair so exp runs as one wide ACT instruction.
  - softmax without max-subtraction (scores are O(1) here): exp on ACT
    (scale=1/8 folded in); causal handled by skipping blocks, one
    triangular-mask multiply on diagonal 128x128 subtiles, and partial-N
    P@V matmuls on diagonal tiles (upper region never read).
  - P@V with V' = [V | ones] stationary (M=65): row 64 accumulates the
    softmax denominator for free; normalize via reciprocal + gpsimd
    partition_broadcast + one DVE multiply.
  - attention runs sq-chunk-outer so the output projection of finished
    chunks (PE-heavy) overlaps the exp-paced attention of later chunks.
"""

import functools

import numpy as np
import ml_dtypes

import concourse.bass as bass
import concourse.mybir as mybir
from concourse import bacc
from concourse.tile import TileContext
from concourse.bass_utils import run_bass_kernel_spmd
from concourse.masks import make_identity

F32 = mybir.dt.float32
BF16 = mybir.dt.bfloat16

DIM = 2048
SEQ = 2048
N_HEADS = 32
N_KV = 8
HD = 64
ROPE_BASE = 10000.0
BATCH = 2
N_CORES = 8

NHC = 8          # q heads per core
NKVC = 2         # kv heads per core
QC = NHC * HD    # 512 q/attn dim columns per core
KC = NKVC * HD   # 128 kv columns per core
NKT = DIM // 128    # 16 contraction chunks
NST = SEQ // 128    # 16 seq tiles
NSC = SEQ // 512    # 4 seq chunks
NMT = QC // 128     # 4 q dim tiles
SCALE = 1.0 / float(np.sqrt(HD))


def _build():
    nc = bacc.Bacc()
    xT = nc.declare_dram_parameter("xT", [DIM, SEQ], BF16, isOutput=False)
    wq = nc.declare_dram_parameter("wq", [128, NKT * QC], BF16, isOutput=False)
    wk = nc.declare_dram_parameter("wk", [128, NKT * KC], BF16, isOutput=False)
    wv = nc.declare_dram_parameter("wv", [128, NKT * KC], BF16, isOutput=False)
    wo = nc.declare_dram_parameter("wo", [128, NMT * DIM], BF16, isOutput=False)
    cosr = nc.declare_dram_parameter("cosr", [128, SEQ], BF16, isOutput=False)
    sinr = nc.declare_dram_parameter("sinr", [128, SEQ], BF16, isOutput=False)
    trim = nc.declare_dram_parameter("trim", [128, 128], BF16, isOutput=False)
    y = nc.declare_dram_parameter("y", [SEQ, DIM], F32, isOutput=True)

    with TileContext(nc) as tc:
        with (
            tc.tile_pool(name="xp", bufs=NKT) as xp,
            tc.tile_pool(name="wp", bufs=1) as wp,
            tc.tile_pool(name="qk", bufs=1) as qk,        # persistent qT/kT/V'/AT
            tc.tile_pool(name="stage", bufs=2) as stage,  # raw projection staging
            tc.tile_pool(name="rt", bufs=1) as rt,        # rope temps
            tc.tile_pool(name="pex", bufs=5) as pex,      # exp'd P tiles (wide)
            tc.tile_pool(name="nrm", bufs=2) as nrm,      # recip / broadcast
            tc.tile_pool(name="ysb", bufs=2) as ysb,      # output staging (wide)
            tc.tile_pool(name="pp", bufs=2, space="PSUM") as pp,   # wide 2-bank tiles
            tc.tile_pool(name="py", bufs=1, space="PSUM") as py,  # oproj accum
            tc.tile_pool(name="pv", bufs=2, space="PSUM") as pv,   # PV accum + transposes
        ):
            # ---------------- load inputs (small weights first) ----------------
            wk_sb = wp.tile([128, NKT * KC], BF16, tag="wk", name="wk")
            nc.sync.dma_start(out=wk_sb[:], in_=wk[:])
            wv_sb = wp.tile([128, NKT * KC], BF16, tag="wv", name="wv")
            nc.sync.dma_start(out=wv_sb[:], in_=wv[:])
            cos_sb = wp.tile([128, SEQ], BF16, tag="cos", name="cos")
            nc.sync.dma_start(out=cos_sb[:], in_=cosr[:])
            sin_sb = wp.tile([128, SEQ], BF16, tag="sin", name="sin")
            nc.sync.dma_start(out=sin_sb[:], in_=sinr[:])
            trim_sb = wp.tile([128, 128], BF16, tag="trim", name="trim")
            nc.sync.dma_start(out=trim_sb[:], in_=trim[:])
            wq_sb = wp.tile([128, NKT * QC], BF16, tag="wq", name="wq")
            nc.sync.dma_start(out=wq_sb[:], in_=wq[:])
            xt = []
            for kc in range(NKT):
                t = xp.tile([128, SEQ], BF16, tag="xt", name="xt")
                nc.sync.dma_start(out=t[:], in_=xT[kc * 128:(kc + 1) * 128, :])
                xt.append(t)
            wo_sb = wp.tile([128, NMT * DIM], BF16, tag="wo", name="wo")
            nc.sync.dma_start(out=wo_sb[:], in_=wo[:])
            ident = wp.tile([128, 128], BF16, tag="ident", name="ident")
            make_identity(nc, ident[:])

            Exp = mybir.ActivationFunctionType.Exp
            Copy = mybir.ActivationFunctionType.Copy
            MUL = mybir.AluOpType.mult
            ADD = mybir.AluOpType.add
            SUB = mybir.AluOpType.subtract

            def rope(raw, dst, base):
                """dst[base:base+64] = rope(raw[base:base+64]); 32-row ops."""
                e = slice(base, base + 32)
                o = slice(base + 32, base + 64)
                t1 = rt.tile([32, SEQ], BF16, tag="t1", name="t1")
                t2 = rt.tile([32, SEQ], BF16, tag="t2", name="t2")
                nc.vector.tensor_tensor(out=t1[:], in0=raw[e, :], in1=cos_sb[e, :], op=MUL)
                nc.vector.tensor_tensor(out=t2[:], in0=raw[o, :], in1=sin_sb[o, :], op=MUL)
                nc.vector.tensor_tensor(out=dst[e, :], in0=t1[:], in1=t2[:], op=SUB)
                t3 = rt.tile([32, SEQ], BF16, tag="t1", name="t3")
                t4 = rt.tile([32, SEQ], BF16, tag="t2", name="t4")
                nc.vector.tensor_tensor(out=t3[:], in0=raw[e, :], in1=sin_sb[e, :], op=MUL)
                nc.vector.tensor_tensor(out=t4[:], in0=raw[o, :], in1=cos_sb[o, :], op=MUL)
                nc.vector.tensor_tensor(out=dst[o, :], in0=t3[:], in1=t4[:], op=ADD)

            def project(w_sb, cols, dst):
                """dst (128, SEQ) bf16 = (w.T @ xT) for one 128-dim tile."""
                pw = [pp.tile([128, 1024], F32, tag="ps", name="ps") for _ in range(2)]
                for kc in range(NKT):
                    lhsT = w_sb[:, kc * cols:kc * cols + 128]
                    for sc in range(NSC):
                        nc.tensor.matmul(pw[sc // 2][:, (sc % 2) * 512:(sc % 2 + 1) * 512],
                                         lhsT, xt[kc][:, sc * 512:(sc + 1) * 512],
                                         start=(kc == 0), stop=(kc == NKT - 1))
                for h in range(2):
                    nc.scalar.activation(dst[:, h * 1024:(h + 1) * 1024], pw[h][:], Copy)

            # ---------------- K projection + rope ----------------
            kraw = stage.tile([128, SEQ], BF16, tag="kraw", name="kraw", bufs=1)
            project(wk_sb, KC, kraw)
            kt = qk.tile([128, SEQ], BF16, tag="kt", name="kt")
            ktsw = qk.tile([128, SEQ], BF16, tag="ktsw", name="ktsw")
            rope(kraw, kt, 0)    # kv0 at rows 0:64
            rope(kraw, kt, 64)   # kv1 at rows 64:128
            nc.vector.tensor_copy(ktsw[0:64, :], kt[64:128, :])   # kv1 at base 0
            nc.vector.tensor_copy(ktsw[64:128, :], kt[0:64, :])   # kv0 at base 64

            # ---------------- V projection + transpose ----------------
            vtmp = stage.tile([128, SEQ], BF16, tag="vtmp", name="vtmp", bufs=1)
            project(wv_sb, KC, vtmp)
            # V' layout: per sk chunk j: [v_kv0 (64) | 1 | v_kv1 (64) | 1]
            vp = qk.tile([128, NST * 130], BF16, tag="vp", name="vp")
            nc.vector.memset(vp[:], 1.0)
            for j in range(NST):
                tp = pv.tile([128, 128], BF16, tag="pv", name="tps")
                nc.tensor.transpose(tp[:], vtmp[:, j * 128:(j + 1) * 128], ident[:])
                nc.vector.tensor_copy(vp[:, j * 130:j * 130 + 64], tp[:, 0:64])
                nc.vector.tensor_copy(vp[:, j * 130 + 65:j * 130 + 129], tp[:, 64:128])

            # ---------------- Q projection + rope ----------------
            qt = []
            for mt in range(NMT):
                qraw = stage.tile([128, SEQ], BF16, tag="qraw", name="qraw")
                pw = [pp.tile([128, 1024], F32, tag="ps", name="ps") for _ in range(2)]
                for kc in range(NKT):
                    lhsT = wq_sb[:, kc * QC + mt * 128:kc * QC + (mt + 1) * 128]
                    for sc in range(NSC):
                        nc.tensor.matmul(pw[sc // 2][:, (sc % 2) * 512:(sc % 2 + 1) * 512],
                                         lhsT, xt[kc][:, sc * 512:(sc + 1) * 512],
                                         start=(kc == 0), stop=(kc == NKT - 1))
                for h in range(2):
                    nc.scalar.activation(qraw[:, h * 1024:(h + 1) * 1024], pw[h][:], Copy)
                qtile = qk.tile([128, SEQ], BF16, tag=f"qt{mt}", name=f"qt{mt}")
                rope(qraw, qtile, 0)
                rope(qraw, qtile, 64)
                qt.append(qtile)

            # ---------------- attention (sq-chunk outer) + fused oproj ----------------
            at = [qk.tile([128, SEQ], BF16, tag=f"at{p}", name=f"at{p}") for p in range(NMT)]
            for qc in range(NSC):
                sq = slice(qc * 512, (qc + 1) * 512)
                njs = 4 * qc + 4
                for p in range(NMT):      # head pair (2p, 2p+1)
                    kvl = p // 2          # local kv head index
                    ktA = (kt if kvl == 0 else ktsw)      # kv at rows 0:64
                    ktB = (ktsw if kvl == 0 else kt)      # same kv at rows 64:128
                    poA = pv.tile([65, 512], F32, tag="pv", name="poA")
                    poB = pv.tile([65, 512], F32, tag="pv", name="poB")
                    for jp in range(njs // 2):
                        wA = pp.tile([128, 1024], F32, tag="ps", name="wA")
                        wB = pp.tile([128, 1024], F32, tag="ps", name="wB")
                        for t in range(2):
                            j = 2 * jp + t
                            sk = slice(j * 128, (j + 1) * 128)
                            nc.tensor.matmul(wA[:, t * 512:(t + 1) * 512],
                                             ktA[0:64, sk], qt[p][0:64, sq],
                                             start=True, stop=True, tile_position=(0, 0))
                            nc.tensor.matmul(wB[:, t * 512:(t + 1) * 512],
                                             ktB[64:128, sk], qt[p][64:128, sq],
                                             start=True, stop=True, tile_position=(64, 0))
                        pxA = pex.tile([128, 1024], BF16, tag="pex", name="pxA")
                        pxB = pex.tile([128, 1024], BF16, tag="pex", name="pxB")
                        nc.scalar.activation(pxA[:], wA[:], Exp, scale=SCALE)
                        nc.scalar.activation(pxB[:], wB[:], Exp, scale=SCALE)
                        for t in range(2):
                            j = 2 * jp + t
                            d = j - 4 * qc
                            vslice = vp[:, j * 130 + 65 * kvl:j * 130 + 65 * kvl + 65]
                            for px, po in ((pxA, poA), (pxB, poB)):
                                if d >= 0:
                                    diag = slice(t * 512 + d * 128, t * 512 + (d + 1) * 128)
                                    nc.vector.tensor_tensor(out=px[:, diag], in0=px[:, diag],
                                                            in1=trim_sb[:], op=MUL)
                                    nc.tensor.matmul(po[:, d * 128:512], vslice,
                                                     px[:, t * 512 + d * 128:(t + 1) * 512],
                                                     start=(j == 0), stop=(j == njs - 1),
                                                     skip_group_check=True)
                                else:
                                    nc.tensor.matmul(po[:], vslice,
                                                     px[:, t * 512:(t + 1) * 512],
                                                     start=(j == 0), stop=(j == njs - 1),
                                                     skip_group_check=True)
                    # normalize by the accumulated denominator (row 64)
                    for po, base in ((poA, 0), (poB, 64)):
                        rs = nrm.tile([1, 512], F32, tag="rs", name="rs")
                        nc.vector.reciprocal(rs[:], po[64:65, :])
                        rbc = nrm.tile([64, 512], F32, tag="rbc", name="rbc")
                        nc.gpsimd.partition_broadcast(rbc[:], rs[:])
                        nc.vector.tensor_tensor(out=at[p][base:base + 64, sq],
                                                in0=po[0:64, :], in1=rbc[:], op=MUL)
                # oproj for the 4 sq tiles of this finished chunk
                for st in range(4 * qc, 4 * qc + 4):
                    for h in range(2):
                        psy = py.tile([128, 1024], F32, tag="psy", name="psy")
                        for dc in range(NMT):
                            lhsT = at[dc][:, st * 128:(st + 1) * 128]
                            for nb in (2 * h, 2 * h + 1):
                                nc.tensor.matmul(psy[:, (nb % 2) * 512:(nb % 2 + 1) * 512],
                                                 lhsT,
                                                 wo_sb[:, dc * DIM + nb * 512:dc * DIM + (nb + 1) * 512],
                                                 start=(dc == 0), stop=(dc == NMT - 1))
                        yt = ysb.tile([128, 1024], F32, tag="ysb", name="ysb")
                        nc.vector.tensor_copy(yt[:], psy[:])
                        nc.sync.dma_start(
                            out=y[st * 128:(st + 1) * 128, h * 1024:(h + 1) * 1024],
                            in_=yt[:])

    nc.finalize()
    return nc


@functools.lru_cache(maxsize=1)
def _built():
    return _build()


def _host_inputs(x, wq, wk, wv, wo):
    """Per-core input maps (host-side shard + layout prep), all bf16."""
    bf = ml_dtypes.bfloat16

    # deinterleave permutation within each head: [even dims, odd dims]
    perm_head = np.concatenate([np.arange(0, HD, 2), np.arange(1, HD, 2)])

    inv_freq = 1.0 / (ROPE_BASE ** (np.arange(0, HD, 2, dtype=np.float32) / HD))
    pos = np.arange(SEQ, dtype=np.float32)
    freqs = pos[:, None] * inv_freq[None, :]          # (SEQ, 32)
    cos_t = np.cos(freqs).astype(np.float32).T        # (32, SEQ)
    sin_t = np.sin(freqs).astype(np.float32).T
    cosr = np.tile(cos_t, (4, 1)).astype(bf)          # (128, SEQ)
    sinr = np.tile(sin_t, (4, 1)).astype(bf)
    trim = (np.arange(128)[None, :] >= np.arange(128)[:, None]).astype(bf)

    def chunked(w, cols):
        # (DIM, cols) -> (128, NKT*cols) with [p, kc*cols + m] = w[kc*128+p, m]
        return np.ascontiguousarray(
            w.reshape(NKT, 128, cols).transpose(1, 0, 2).reshape(128, NKT * cols))

    in_maps = []
    for c in range(N_CORES):
        b, g = divmod(c, 4)
        heads = np.arange(NHC) + NHC * g
        qcols = (heads[:, None] * HD + perm_head[None, :]).reshape(-1)
        kvh = np.arange(NKVC) + NKVC * g
        kcols = (kvh[:, None] * HD + perm_head[None, :]).reshape(-1)
        vcols = (kvh[:, None] * HD + np.arange(HD)[None, :]).reshape(-1)
        orows = (heads[:, None] * HD + np.arange(HD)[None, :]).reshape(-1)

        xT = np.ascontiguousarray(x[b].T).astype(bf)
        wq_c = chunked(wq[:, qcols].astype(bf), QC)
        wk_c = chunked(wk[:, kcols].astype(bf), KC)
        wv_c = chunked(wv[:, vcols].astype(bf), KC)
        wo_c = np.ascontiguousarray(
            wo[orows, :].astype(bf).reshape(NMT, 128, DIM)
            .transpose(1, 0, 2).reshape(128, NMT * DIM))
        in_maps.append(dict(xT=xT, wq=wq_c, wk=wk_c, wv=wv_c, wo=wo_c,
                            cosr=cosr, sinr=sinr, trim=trim))
    return in_maps


def kernel(x, wq, wk, wv, wo):
    nc = _built()
    in_maps = _host_inputs(np.asarray(x, np.float32), np.asarray(wq, np.float32),
                           np.asarray(wk, np.float32), np.asarray(wv, np.float32),
                           np.asarray(wo, np.float32))
    res = run_bass_kernel_spmd(nc, in_maps, core_ids=list(range(N_CORES)))
    outs = [res.results[c]["y"] for c in range(N_CORES)]
    y = np.stack([outs[4 * b] + outs[4 * b + 1] + outs[4 * b + 2] + outs[4 * b + 3]
                  for b in range(BATCH)], axis=0)
    return y.astype(np.float32)
